# revision 27
# baseline (speedup 1.0000x reference)
"""DGCNN edge-conv block on 8 Trainium2 NeuronCores.

Sharding: data-parallel over (batch, query-half): core i handles batch i//2,
queries [2048*(i%2) : +2048] of that batch's 4096 points. Each core gets the
full point cloud of its batch (keys) with columns permuted so its own queries
are always columns 0..2047 (SPMD: one program, per-core inputs).

Numerics: the KNN scores use a single fp16 matmul pass (fp32 PSUM):
s/2 = x16.x16 - xx/2. The output is invariant to the ORDER of the top-3
(max over k downstream), only the selected set matters; fp16 operand
rounding perturbs scores by ~0.01 while the typical 3rd/4th-neighbor gap
is ~10, so only ~20 of the 16384 query sets flip vs the fp32 reference
(measured: final rel err ~1e-2, gate 2e-2). xx is reduced from fp16
squares on the PE (ones-vector matmul); the -xx/2 per-key offset is
folded into the same PSUM group as a K=2 matmul of a 2-way fp16 split
(split error ~1e-4). Conv weights/activations use fp16 (values only, no
selection; fp32 PSUM), final BN+ReLU writes fp32.

Pipeline per core:
  A: load x16; xx (fp16 squares, PE reduce); A=w1n@x, Bv=w1c@x_q -> DRAM
  B: per (query-tile, key-tile): 8 fp16 matmuls + xx-fold; top-8 via
     DVE max/max_index; top-3 indices -> DRAM in ap_gather wrapped layout
  C: gather A columns (gpsimd ap_gather), + Bv, BN+ReLU -> h1 (fp16)
  D: conv2..conv4 with max-over-k, cat, conv5 -> out [1024, 2048] fp32
"""

import sys

sys.path.insert(0, "/opt/trn_rl_repo")

import numpy as np

B, C_IN, N, K = 4, 1000, 4096, 3
CPAD = 1024        # padded contraction dim
NQ = 2048          # queries per core
CP = 128           # contraction chunk partitions
CH = 8             # number of contraction chunks
NT = 512           # key tile (psum bank width in fp32)
NNT = N // NT      # 8 key tiles
QT = 128           # query tile (psum partitions)
NQT = NQ // QT     # 16 query tiles
SEG = 512          # conv-phase query segment
NSEG = NQ // SEG   # 4 segments
EPS = np.float32(1e-5)

_CACHE = {}


def build_nc(finalize=True):
    import concourse.mybir as mybir
    import concourse.tile as tile
    from concourse import bacc

    f32 = mybir.dt.float32
    f16 = mybir.dt.float16
    bf16 = mybir.dt.bfloat16
    u16 = mybir.dt.uint16
    i16 = mybir.dt.int16
    Relu = mybir.ActivationFunctionType.Relu

    nc = bacc.Bacc("TRN2", target_bir_lowering=False, debug=False, num_devices=8)

    xh = nc.dram_tensor("xh", [CPAD, N], f16, kind="ExternalInput").ap()
    w1t = nc.dram_tensor("w1t", [CPAD, 128], f16, kind="ExternalInput").ap()
    w2t = nc.dram_tensor("w2t", [64, 128], f16, kind="ExternalInput").ap()
    w3t = nc.dram_tensor("w3t", [128, 256], f16, kind="ExternalInput").ap()
    w4t = nc.dram_tensor("w4t", [256, 512], f16, kind="ExternalInput").ap()
    w5p = nc.dram_tensor("w5p", [128, 8, 1024], f16, kind="ExternalInput").ap()
    sb1 = nc.dram_tensor("sb1", [64, 2], f32, kind="ExternalInput").ap()
    sb2 = nc.dram_tensor("sb2", [128, 2], f32, kind="ExternalInput").ap()
    sb3 = nc.dram_tensor("sb3", [128, 4], f32, kind="ExternalInput").ap()
    sb4 = nc.dram_tensor("sb4", [128, 8], f32, kind="ExternalInput").ap()
    sb5 = nc.dram_tensor("sb5", [128, 16], f32, kind="ExternalInput").ap()
    out = nc.dram_tensor("out", [1024, NQ], f32, kind="ExternalOutput").ap()

    with tile.TileContext(nc) as tc:
        _body(nc, tc, mybir, xh, w1t, w2t, w3t, w4t, w5p,
              sb1, sb2, sb3, sb4, sb5, out, f32, f16, bf16, u16, i16, Relu)
    if finalize:
        nc.finalize()
    return nc


def _body(nc, tc, mybir, xh, w1t, w2t, w3t, w4t, w5p,
          sb1, sb2, sb3, sb4, sb5, out, f32, f16, bf16, u16, i16, Relu):
    from contextlib import ExitStack
    from concourse import library_config

    NQH = 1024           # xx quarter width
    es = ExitStack()
    with es:
        p_c1 = es.enter_context(tc.tile_pool(name="c1", bufs=1))

        # gpsimd library for the gathers; dummy gather + drain force the
        # ucode load now so it overlaps the early phases.
        nc.gpsimd.load_library(library_config.ap_gather)
        dmy = p_c1.tile([64, 16], f32, tag="dmy")
        dmys = p_c1.tile([64, 4], f32, tag="dmys")
        dmyi = p_c1.tile([64, 1], i16, tag="dmyi")
        nc.vector.memset(dmys[:], 0.0)
        nc.vector.memset(dmyi[:], 0)
        nc.gpsimd.ap_gather(out_ap=dmy[:], in_ap=dmys[:], idxs_ap=dmyi[:],
                            channels=64, num_elems=4, d=1, num_idxs=16)
        nc.gpsimd.drain()

        # ---- persistent small tensors ----
        w1s = p_c1.tile([CP, CH, 128], f16, tag="w1s")
        nc.sync.dma_start(w1s[:], w1t.rearrange("(c p) m -> p c m", p=CP))
        ones_col = p_c1.tile([128, 1], f16, tag="ones_col")
        nc.vector.memset(ones_col[:], 1.0)
        ones2 = p_c1.tile([2, NQ], f16, tag="ones2")
        nc.vector.memset(ones2[:], 1.0)
        # candidate index offsets: offs64[:, nt*8+k] = nt*512
        offs64 = p_c1.tile([128, 64], f32, tag="offs64")
        for _nt in range(NNT):
            nc.vector.memset(offs64[:, _nt * 8:(_nt + 1) * 8], float(_nt * NT))
        # top-3 indices: stage3[p, qt, kk] = idx_kk(qt*128 + p)
        stage3 = p_c1.tile([128, NQT, 3], u16, tag="stage3")
        # h1 pre-activation, fp16, kk-major q-ordered [64, 3*NQ]
        h1pre = p_c1.tile([64, 3 * NQ], f16, tag="h1pre")
        Bv = p_c1.tile([64, NQ], f32, tag="Bv")

        # x: quarter-granular loads so phase-A quarter 0 starts after ~2MB
        xhs = p_c1.tile([CP, CH, N], f16, tag="xhs")
        xhr = xh.rearrange("(c p) n -> p c n", p=CP)
        for q4 in range(N // NQH):
            for c in range(CH):
                qsl = slice(q4 * NQH, (q4 + 1) * NQH)
                nc.sync.dma_start(xhs[:, c, qsl], xhr[:, c, qsl])
        # chunk-7 key-side pad partitions 104/105 will hold the 2-way fp16
        # split of -xx/2; the query-side stationary xq7 is the same chunk
        # with ones in those partitions, so the per-key xx offset rides
        # along chunk 7's matmul (no separate fold matmul).
        xq7 = p_c1.tile([CP, NQ], f16, tag="xq7")
        nc.sync.dma_start(xq7[:], xhs[:, CH - 1, 0:NQ])
        nc.sync.dma_start(xq7[104:106, :], ones2[:, :])
        A = p_c1.tile([64, N], f32, tag="A")

        # conv weights/BN up-front (never gates the conv phase)
        w2s = p_c1.tile([64, 128], f16, tag="w2s")
        nc.sync.dma_start(w2s[:], w2t[:])
        w3s = p_c1.tile([128, 256], f16, tag="w3s")
        nc.sync.dma_start(w3s[:], w3t[:])
        w4s = p_c1.tile([128, 2, 512], f16, tag="w4s")
        nc.sync.dma_start(w4s[:], w4t.rearrange("(c p) m -> p c m", p=128))
        w5s = p_c1.tile([128, 8, 1024], f16, tag="w5s")
        nc.sync.dma_start(w5s[:], w5p[:])
        sb1s = p_c1.tile([64, 2], f32, tag="sb1s")
        nc.sync.dma_start(sb1s[:], sb1[:])
        sb2s = p_c1.tile([128, 2], f32, tag="sb2s")
        nc.sync.dma_start(sb2s[:], sb2[:])
        sb3s = p_c1.tile([128, 4], f32, tag="sb3s")
        nc.sync.dma_start(sb3s[:], sb3[:])
        sb4s = p_c1.tile([128, 8], f32, tag="sb4s")
        nc.sync.dma_start(sb4s[:], sb4[:])
        sb5s = p_c1.tile([128, 16], f32, tag="sb5s")
        nc.sync.dma_start(sb5s[:], sb5[:])

        h1 = h1pre.rearrange("p (k q) -> p k q", k=3)  # [64, 3, NQ] fp16
        outr = out.rearrange("(c p) n -> p c n", p=128)

        # ---- phase A: xx + A/Bv in 4 quarter-passes ----
        with nc.named_scope("prep"):
            with tc.tile_pool(name="pa1", bufs=1) as p_a1, \
                 tc.tile_pool(name="psxx", bufs=4, space="PSUM") as p_psxx, \
                 tc.tile_pool(name="psa", bufs=4, space="PSUM") as p_psa:
                # all squares hoisted: DVE never blocks them behind the
                # per-quarter mxx split chains
                xsqs = {}
                for q4 in range(N // NQH):
                    qsl = slice(q4 * NQH, (q4 + 1) * NQH)
                    for c in range(CH):
                        xsq = p_a1.tile([CP, NQH], f16, tag="xsq", bufs=6)
                        nc.vector.tensor_mul(xsq[:], xhs[:, c, qsl],
                                             xhs[:, c, qsl])
                        xsqs[q4, c] = xsq
                for q4 in range(N // NQH):
                    qsl = slice(q4 * NQH, (q4 + 1) * NQH)
                    nts = [q4 * 2, q4 * 2 + 1]
                    psx = [p_psxx.tile([1, NT], f32, tag="psxx",
                                       name=f"psxx{q4}_{_j}")
                           for _j in range(2)]
                    pav = [p_psa.tile([128, NT], f32, tag="pa",
                                      name=f"pa{q4}_{_j}")
                           for _j in range(2)]
                    for c in range(CH):
                        xsq = xsqs[q4, c]
                        for j, nt in enumerate(nts):
                            mw = 128 if nt < NQ // NT else 64
                            nc.tensor.matmul(
                                psx[j][:], ones_col[:, :],
                                xsq[:, j * NT:(j + 1) * NT],
                                start=(c == 0), stop=(c == CH - 1))
                            nc.tensor.matmul(
                                pav[j][0:mw, :], w1s[:, c, 0:mw],
                                xhs[:, c, q4 * NQH + j * NT:
                                    q4 * NQH + (j + 1) * NT],
                                start=(c == 0), stop=(c == CH - 1))
                    mxxq = p_a1.tile([1, NQH], f32, tag="mxxq")
                    msp = p_a1.tile([1, NQH], f16, tag="msp")
                    for j, nt in enumerate(nts):
                        nc.scalar.mul(mxxq[:, j * NT:(j + 1) * NT],
                                      psx[j][:], -0.5)
                        nc.scalar.copy(A[:, nt * NT:(nt + 1) * NT],
                                       pav[j][0:64, :])
                        if nt < NQ // NT:
                            nc.scalar.copy(Bv[:, nt * NT:(nt + 1) * NT],
                                           pav[j][64:128, :])
                    # 2-way fp16 split of -xx/2 (split error ~1e-4) into
                    # chunk-7 pad partitions 104/105 (via DMA: engine
                    # start-partition must be 0/32/64/96)
                    for j in range(2):
                        nc.scalar.copy(msp[:], mxxq[:])
                        nc.sync.dma_start(
                            xhs[104 + j:105 + j, CH - 1, qsl], msp[:])
                        if j < 1:
                            nc.vector.tensor_sub(mxxq[:], mxxq[:], msp[:])

        # ---- phases B (knn) + D (convs) interleaved per segment ----
        with tc.tile_pool(name="pss", bufs=6, space="PSUM") as p_pss, \
             tc.tile_pool(name="psd", bufs=2, space="PSUM") as p_psd, \
             tc.tile_pool(name="ms", bufs=1) as p_s, \
             tc.tile_pool(name="m8", bufs=2) as p_m8, \
             tc.tile_pool(name="gq", bufs=2) as p_gq, \
             tc.tile_pool(name="seg", bufs=1) as p_seg, \
             tc.tile_pool(name="tmp", bufs=2) as p_tmp, \
             tc.tile_pool(name="osb", bufs=4) as p_osb:
            idxw = p_s.tile([64, 3 * NQ // 16], i16, tag="idxw")

            def knn_qt(qt):
                qs = slice(qt * QT, (qt + 1) * QT)
                # per key-tile top-8 straight from PSUM (overlaps the
                # remaining matmuls), then a 64-candidate merge: the
                # serial tail is ~1us instead of a 4.4us full-row scan
                m64 = p_m8.tile([QT, 64], f32, tag="m64")
                i64 = p_m8.tile([QT, 64], u16, tag="i64")
                for nt in range(NNT):
                    ns = slice(nt * NT, (nt + 1) * NT)
                    ps = p_pss.tile([QT, NT], f32, tag="pss")
                    for c in range(CH):
                        stat = xq7[:, qs] if c == CH - 1 else xhs[:, c, qs]
                        nc.tensor.matmul(ps[:], stat, xhs[:, c, ns],
                                         start=(c == 0), stop=(c == CH - 1))
                    c8 = slice(nt * 8, (nt + 1) * 8)
                    nc.vector.max(out=m64[:, c8], in_=ps[:])
                    nc.vector.max_index(out=i64[:, c8], in_max=m64[:, c8],
                                        in_values=ps[:])
                # global candidate indices as f32 values
                g64 = p_m8.tile([QT, 64], f32, tag="g64")
                nc.vector.tensor_copy(g64[:], i64[:])
                nc.vector.tensor_add(g64[:], g64[:], offs64[:])
                m8 = p_m8.tile([QT, 8], f32, tag="m8")
                nc.vector.max(out=m8[:], in_=m64[:])
                # per-partition index extraction: for each top-k value,
                # idx = sum((m64 == v_k) * g64) over the 64 candidates
                idx3 = p_m8.tile([QT, 3], f32, tag="idx3")
                eqm = p_m8.tile([QT, 64], f32, tag="eqm")
                for kk in range(3):
                    nc.vector.scalar_tensor_tensor(
                        out=eqm[:], in0=m64[:], scalar=m8[:, kk:kk + 1],
                        in1=g64[:], op0=mybir.AluOpType.is_equal,
                        op1=mybir.AluOpType.mult,
                        accum_out=idx3[:, kk:kk + 1])
                nc.vector.tensor_copy(stage3[:, qt, :], idx3[:])
                # wrap this qt's indices into ap_gather layout:
                # idxw[16*g2+r, qt*24 + g*3 + kk] = stage3[g*16+r, qt, kk]
                # (one wave of independent small DMAs: no serial
                # replication round-trip on the critical chain)
                for g2 in range(4):
                    for g in range(8):
                        nc.sync.dma_start(
                            idxw[16 * g2:16 * (g2 + 1),
                                 qt * 24 + 3 * g:qt * 24 + 3 * g + 3],
                            stage3[16 * g:16 * (g + 1), qt, :].bitcast(i16))
                # gather this qt's neighbor features (overlaps the
                # remaining distance matmuls on the Q7s);
                # gather position 16*(g*3+kk)+r = query g*16+r
                gq = p_gq.tile([64, 3 * QT], f32, tag="gq")
                nc.gpsimd.ap_gather(
                    out_ap=gq[:], in_ap=A[:],
                    idxs_ap=idxw[:, qt * 24:(qt + 1) * 24],
                    channels=64, num_elems=N, d=1, num_idxs=3 * QT)
                # unpermute into h1pre (fp16, kk-major q-order)
                gqv = gq.rearrange("p (g kk r) -> p g kk r",
                                   g=8, kk=3, r=16)
                hv = h1pre.rearrange("p (kk q) -> p kk q", kk=3)
                dst = hv[:, :, qt * QT:(qt + 1) * QT] \
                    .rearrange("p kk (g r) -> p g kk r", g=8)
                nc.scalar.copy(dst, gqv[:])
                # h1 BN per qt (off the seg-boundary critical chain)
                h1_seg(qt * QT, QT)

            def h1_seg(q0, w):
                # h1 = relu(bn1(h1pre + Bv)) in place for this segment
                qs = slice(q0, q0 + w)
                bvb = Bv[:, qs].unsqueeze(1).to_broadcast([64, 3, w])
                nc.vector.tensor_add(h1[:, :, qs], h1[:, :, qs], bvb)
                nc.scalar.activation(h1[:, :, qs], h1[:, :, qs], Relu,
                                     bias=sb1s[:, 1:2], scale=sb1s[:, 0:1])

            def conv_seg(q0, w):
                qs = slice(q0, q0 + w)
                h2 = p_seg.tile([128, 3, w], f16, tag="h2")
                h3 = p_seg.tile([128, 2, 3, w], f16, tag="h3")
                h4 = p_seg.tile([128, 4, 3, w], f16, tag="h4")
                cat = p_seg.tile([128, 8, w], f16, tag="cat")
                nc.vector.memset(cat[64:128, 0, :], 0.0)

                # conv2 (K=64 -> 128)
                for kk in range(3):
                    ps2 = p_psd.tile([128, w], f32, tag="psd")
                    nc.tensor.matmul(ps2[:], w2s[:], h1[:, kk, qs],
                                     start=True, stop=True)
                    nc.scalar.activation(h2[:, kk, :], ps2[:], Relu,
                                         bias=sb2s[:, 1:2], scale=sb2s[:, 0:1])
                # x1 -> cat chunk 0 (64 rows)
                t1 = p_tmp.tile([64, w], f16, tag="t64")
                nc.vector.tensor_max(t1[:], h1[:, 1, qs], h1[:, 2, qs])
                nc.vector.tensor_max(cat[0:64, 0, :], t1[:], h1[:, 0, qs])
                # x2 -> cat chunk 1
                t2 = p_tmp.tile([128, w], f16, tag="t128")
                nc.vector.tensor_max(t2[:], h2[:, 1, :], h2[:, 2, :])
                nc.vector.tensor_max(cat[:, 1, :], t2[:], h2[:, 0, :])

                # conv3 (K=128 -> 256 in 2 chunks)
                for m in range(2):
                    for kk in range(3):
                        ps3 = p_psd.tile([128, w], f32, tag="psd")
                        nc.tensor.matmul(ps3[:], w3s[:, m * 128:(m + 1) * 128],
                                         h2[:, kk, :], start=True, stop=True)
                        nc.scalar.activation(h3[:, m, kk, :], ps3[:], Relu,
                                             bias=sb3s[:, 2 + m:3 + m],
                                             scale=sb3s[:, m:m + 1])
                # x3 -> cat chunks 2,3
                for m in range(2):
                    t3 = p_tmp.tile([128, w], f16, tag="t128")
                    nc.vector.tensor_max(t3[:], h3[:, m, 1, :], h3[:, m, 2, :])
                    nc.vector.tensor_max(cat[:, 2 + m, :], t3[:], h3[:, m, 0, :])

                # conv4 (K=256 in 2 chunks -> 512 in 4 chunks)
                for m in range(4):
                    for kk in range(3):
                        ps4 = p_psd.tile([128, w], f32, tag="psd")
                        for c in range(2):
                            nc.tensor.matmul(
                                ps4[:], w4s[:, c, m * 128:(m + 1) * 128],
                                h3[:, c, kk, :], start=(c == 0), stop=(c == 1))
                        nc.scalar.activation(h4[:, m, kk, :], ps4[:], Relu,
                                             bias=sb4s[:, 4 + m:5 + m],
                                             scale=sb4s[:, m:m + 1])
                # x4 -> cat chunks 4..7
                for m in range(4):
                    t4 = p_tmp.tile([128, w], f16, tag="t128")
                    nc.vector.tensor_max(t4[:], h4[:, m, 1, :], h4[:, m, 2, :])
                    nc.vector.tensor_max(cat[:, 4 + m, :], t4[:], h4[:, m, 0, :])

                # conv5 (K=960 padded to 8*128 -> 1024 in 8 chunks)
                for m in range(8):
                    ps5 = p_psd.tile([128, w], f32, tag="psd")
                    for c in range(8):
                        nc.tensor.matmul(
                            ps5[:], w5s[:, c, m * 128:(m + 1) * 128],
                            cat[:, c, :], start=(c == 0), stop=(c == 7))
                    o5 = p_osb.tile([128, w], f32, tag="o5")
                    nc.scalar.activation(o5[:], ps5[:], Relu,
                                         bias=sb5s[:, 8 + m:9 + m],
                                         scale=sb5s[:, m:m + 1])
                    nc.sync.dma_start(outr[:, m, qs], o5[:])

            # emission schedule: conv strips one qt behind their last
            # query tile; the last segment runs as two 256-col strips to
            # shrink the serial tail after the final knn tile
            conv_at = {4: (0, SEG), 8: (SEG, SEG), 12: (2 * SEG, SEG),
                       14: (3 * SEG, SEG // 2)}
            for qt in range(NQT):
                with nc.named_scope("knn"):
                    knn_qt(qt)
                if qt in conv_at:
                    with nc.named_scope("convs"):
                        conv_seg(*conv_at[qt])
            with nc.named_scope("convs"):
                conv_seg(3 * SEG + SEG // 2, SEG // 2)


def prep_inputs(inputs):
    """Host-side sharding + layout/precision prep. Returns per-core in_maps."""
    x = np.ascontiguousarray(inputs["x"], dtype=np.float32)  # [B, C, N]
    shared = {}
    w1 = inputs["w1"].astype(np.float32)
    w1p = np.zeros((CPAD, 128), dtype=np.float16)
    w1p[:C_IN, 0:64] = w1[:, :C_IN].T.astype(np.float16)
    w1p[:C_IN, 64:128] = w1[:, C_IN:].T.astype(np.float16)
    shared["w1t"] = w1p
    shared["w2t"] = np.ascontiguousarray(inputs["w2"].T.astype(np.float16))
    shared["w3t"] = np.ascontiguousarray(inputs["w3"].T.astype(np.float16))
    shared["w4t"] = np.ascontiguousarray(inputs["w4"].T.astype(np.float16))
    w5t = inputs["w5"].astype(np.float32).T  # [960, 1024]
    w5p = np.zeros((128, 8, 1024), dtype=np.float16)
    w5p[0:64, 0, :] = w5t[0:64]          # x1 block
    w5p[:, 1, :] = w5t[64:192]           # x2
    w5p[:, 2, :] = w5t[192:320]          # x3 lo
    w5p[:, 3, :] = w5t[320:448]          # x3 hi
    for m in range(4):                   # x4
        w5p[:, 4 + m, :] = w5t[448 + 128 * m:448 + 128 * (m + 1)]
    shared["w5p"] = w5p

    def scale_bias(i):
        g = inputs[f"g{i}"].astype(np.float32)
        b = inputs[f"b{i}"].astype(np.float32)
        m = inputs[f"m{i}"].astype(np.float32)
        v = inputs[f"v{i}"].astype(np.float32)
        s = g / np.sqrt(v + EPS)
        return s.astype(np.float32), (b - m * s).astype(np.float32)

    s1, b1 = scale_bias(1)
    shared["sb1"] = np.ascontiguousarray(np.stack([s1, b1], axis=1))
    s2, b2 = scale_bias(2)
    shared["sb2"] = np.ascontiguousarray(np.stack([s2, b2], axis=1))
    s3, b3 = scale_bias(3)
    shared["sb3"] = np.ascontiguousarray(
        np.stack([s3[:128], s3[128:], b3[:128], b3[128:]], axis=1))
    s4, b4 = scale_bias(4)
    shared["sb4"] = np.ascontiguousarray(np.stack(
        [s4[128 * m:128 * (m + 1)] for m in range(4)]
        + [b4[128 * m:128 * (m + 1)] for m in range(4)], axis=1))
    s5, b5 = scale_bias(5)
    shared["sb5"] = np.ascontiguousarray(np.stack(
        [s5[128 * m:128 * (m + 1)] for m in range(8)]
        + [b5[128 * m:128 * (m + 1)] for m in range(8)], axis=1))

    in_maps = []
    for core in range(8):
        b, half = core // 2, core % 2
        q0 = half * NQ
        other0 = NQ - q0  # 2048 if half==0 else 0
        xbp = np.concatenate([x[b][:, q0:q0 + NQ], x[b][:, other0:other0 + NQ]],
                             axis=1)  # [1000, 4096], own queries first
        xpad = np.zeros((CPAD, N), dtype=np.float16)
        xpad[:C_IN] = xbp.astype(np.float16)
        m = dict(shared)
        m["xh"] = xpad
        in_maps.append(m)
    return in_maps


def kernel(**inputs):
    from concourse.bass_utils import run_bass_kernel_spmd

    if "nc" not in _CACHE:
        _CACHE["nc"] = build_nc()
    nc = _CACHE["nc"]
    in_maps = prep_inputs(inputs)
    res = run_bass_kernel_spmd(nc, in_maps, core_ids=list(range(8)))
    out = np.empty((B, 1024, N), dtype=np.float32)
    for core in range(8):
        b, half = core // 2, core % 2
        q0 = half * NQ
        out[b, :, q0:q0 + NQ] = res.results[core]["out"]
    return out



# revision 30
# speedup vs baseline: 1.0412x; 1.0412x over previous
"""DGCNN edge-conv block on 8 Trainium2 NeuronCores.

Sharding: data-parallel over (batch, query-half): core i handles batch i//2,
queries [2048*(i%2) : +2048] of that batch's 4096 points. Each core gets the
full point cloud of its batch (keys) with columns permuted so its own queries
are always columns 0..2047 (SPMD: one program, per-core inputs).

Numerics: the KNN scores use a single fp16 matmul pass (fp32 PSUM):
s/2 = x16.x16 - xx/2. The output is invariant to the ORDER of the top-3
(max over k downstream), only the selected set matters; fp16 operand
rounding perturbs scores by ~0.01 while the typical 3rd/4th-neighbor gap
is ~10, so only ~20 of the 16384 query sets flip vs the fp32 reference
(measured: final rel err ~1e-2, gate 2e-2). xx is reduced from fp16
squares on the PE (ones-vector matmul); the -xx/2 per-key offset is
folded into the same PSUM group as a K=2 matmul of a 2-way fp16 split
(split error ~1e-4). Conv weights/activations use fp16 (values only, no
selection; fp32 PSUM), final BN+ReLU writes fp32.

Pipeline per core:
  A: load x16; xx (fp16 squares, PE reduce); A=w1n@x, Bv=w1c@x_q -> DRAM
  B: per (query-tile, key-tile): 8 fp16 matmuls + xx-fold; top-8 via
     DVE max/max_index; top-3 indices -> DRAM in ap_gather wrapped layout
  C: gather A columns (gpsimd ap_gather), + Bv, BN+ReLU -> h1 (fp16)
  D: conv2..conv4 with max-over-k, cat, conv5 -> out [1024, 2048] fp32
"""

import sys

sys.path.insert(0, "/opt/trn_rl_repo")

import numpy as np

B, C_IN, N, K = 4, 1000, 4096, 3
CPAD = 1024        # padded contraction dim
NQ = 2048          # queries per core
CP = 128           # contraction chunk partitions
CH = 8             # number of contraction chunks
NT = 512           # key tile (psum bank width in fp32)
NNT = N // NT      # 8 key tiles
QT = 128           # query tile (psum partitions)
NQT = NQ // QT     # 16 query tiles
SEG = 512          # conv-phase query segment
NSEG = NQ // SEG   # 4 segments
EPS = np.float32(1e-5)

_CACHE = {}


def build_nc(finalize=True):
    import concourse.mybir as mybir
    import concourse.tile as tile
    from concourse import bacc

    f32 = mybir.dt.float32
    f16 = mybir.dt.float16
    bf16 = mybir.dt.bfloat16
    u16 = mybir.dt.uint16
    i16 = mybir.dt.int16
    Relu = mybir.ActivationFunctionType.Relu

    nc = bacc.Bacc("TRN2", target_bir_lowering=False, debug=False, num_devices=8)

    xh = nc.dram_tensor("xh", [CPAD, N], f16, kind="ExternalInput").ap()
    w1t = nc.dram_tensor("w1t", [CPAD, 128], f16, kind="ExternalInput").ap()
    w2t = nc.dram_tensor("w2t", [64, 128], f16, kind="ExternalInput").ap()
    w3t = nc.dram_tensor("w3t", [128, 256], f16, kind="ExternalInput").ap()
    w4t = nc.dram_tensor("w4t", [256, 512], f16, kind="ExternalInput").ap()
    w5p = nc.dram_tensor("w5p", [128, 8, 1024], f16, kind="ExternalInput").ap()
    sb1 = nc.dram_tensor("sb1", [64, 2], f32, kind="ExternalInput").ap()
    sb2 = nc.dram_tensor("sb2", [128, 2], f32, kind="ExternalInput").ap()
    sb3 = nc.dram_tensor("sb3", [128, 4], f32, kind="ExternalInput").ap()
    sb4 = nc.dram_tensor("sb4", [128, 8], f32, kind="ExternalInput").ap()
    sb5 = nc.dram_tensor("sb5", [128, 16], f32, kind="ExternalInput").ap()
    out = nc.dram_tensor("out", [1024, NQ], f32, kind="ExternalOutput").ap()

    with tile.TileContext(nc) as tc:
        _body(nc, tc, mybir, xh, w1t, w2t, w3t, w4t, w5p,
              sb1, sb2, sb3, sb4, sb5, out, f32, f16, bf16, u16, i16, Relu)
    if finalize:
        nc.finalize()
    return nc


def _body(nc, tc, mybir, xh, w1t, w2t, w3t, w4t, w5p,
          sb1, sb2, sb3, sb4, sb5, out, f32, f16, bf16, u16, i16, Relu):
    from contextlib import ExitStack
    from concourse import library_config

    NQH = 1024           # xx quarter width
    es = ExitStack()
    with es:
        p_c1 = es.enter_context(tc.tile_pool(name="c1", bufs=1))

        # gpsimd library for the gathers; dummy gather + drain force the
        # ucode load now so it overlaps the early phases.
        nc.gpsimd.load_library(library_config.ap_gather)
        dmy = p_c1.tile([64, 16], f32, tag="dmy")
        dmys = p_c1.tile([64, 4], f32, tag="dmys")
        dmyi = p_c1.tile([64, 1], i16, tag="dmyi")
        nc.vector.memset(dmys[:], 0.0)
        nc.vector.memset(dmyi[:], 0)
        nc.gpsimd.ap_gather(out_ap=dmy[:], in_ap=dmys[:], idxs_ap=dmyi[:],
                            channels=64, num_elems=4, d=1, num_idxs=16)
        nc.gpsimd.drain()

        # ---- persistent small tensors ----
        w1s = p_c1.tile([CP, CH, 128], f16, tag="w1s")
        nc.sync.dma_start(w1s[:], w1t.rearrange("(c p) m -> p c m", p=CP))
        ones_col = p_c1.tile([128, 1], f16, tag="ones_col")
        nc.vector.memset(ones_col[:], 1.0)
        ones2 = p_c1.tile([2, NQ], f16, tag="ones2")
        nc.vector.memset(ones2[:], 1.0)
        # candidate index offsets: offs64[:, nt*8+k] = nt*512
        offs64 = p_c1.tile([128, 64], f32, tag="offs64")
        for _nt in range(NNT):
            nc.vector.memset(offs64[:, _nt * 8:(_nt + 1) * 8], float(_nt * NT))
        # top-3 indices: stage3[p, qt, kk] = idx_kk(qt*128 + p)
        stage3 = p_c1.tile([128, NQT, 3], u16, tag="stage3")
        # h1 pre-activation, fp16, kk-major q-ordered [64, 3*NQ]
        h1pre = p_c1.tile([64, 3 * NQ], f16, tag="h1pre")
        Bv = p_c1.tile([64, NQ], f32, tag="Bv")

        # x: quarter-granular loads so phase-A quarter 0 starts after ~2MB
        xhs = p_c1.tile([CP, CH, N], f16, tag="xhs")
        xhr = xh.rearrange("(c p) n -> p c n", p=CP)
        for q4 in range(N // NQH):
            for c in range(CH):
                qsl = slice(q4 * NQH, (q4 + 1) * NQH)
                nc.sync.dma_start(xhs[:, c, qsl], xhr[:, c, qsl])
        # chunk-7 key-side pad partitions 104/105 will hold the 2-way fp16
        # split of -xx/2; the query-side stationary xq7 is the same chunk
        # with ones in those partitions, so the per-key xx offset rides
        # along chunk 7's matmul (no separate fold matmul).
        xq7 = p_c1.tile([CP, NQ], f16, tag="xq7")
        nc.sync.dma_start(xq7[:], xhs[:, CH - 1, 0:NQ])
        nc.sync.dma_start(xq7[104:106, :], ones2[:, :])
        A = p_c1.tile([64, N], f32, tag="A")

        # conv weights/BN up-front (never gates the conv phase)
        w2s = p_c1.tile([64, 128], f16, tag="w2s")
        nc.sync.dma_start(w2s[:], w2t[:])
        w3s = p_c1.tile([128, 256], f16, tag="w3s")
        nc.sync.dma_start(w3s[:], w3t[:])
        w4s = p_c1.tile([128, 2, 512], f16, tag="w4s")
        nc.sync.dma_start(w4s[:], w4t.rearrange("(c p) m -> p c m", p=128))
        w5s = p_c1.tile([128, 8, 1024], f16, tag="w5s")
        nc.sync.dma_start(w5s[:], w5p[:])
        sb1s = p_c1.tile([64, 2], f32, tag="sb1s")
        nc.sync.dma_start(sb1s[:], sb1[:])
        sb2s = p_c1.tile([128, 2], f32, tag="sb2s")
        nc.sync.dma_start(sb2s[:], sb2[:])
        sb3s = p_c1.tile([128, 4], f32, tag="sb3s")
        nc.sync.dma_start(sb3s[:], sb3[:])
        sb4s = p_c1.tile([128, 8], f32, tag="sb4s")
        nc.sync.dma_start(sb4s[:], sb4[:])
        sb5s = p_c1.tile([128, 16], f32, tag="sb5s")
        nc.sync.dma_start(sb5s[:], sb5[:])

        h1 = h1pre.rearrange("p (k q) -> p k q", k=3)  # [64, 3, NQ] fp16
        outr = out.rearrange("(c p) n -> p c n", p=128)

        # ---- phase A: xx + A/Bv in 4 quarter-passes ----
        with nc.named_scope("prep"):
            with tc.tile_pool(name="pa1", bufs=1) as p_a1, \
                 tc.tile_pool(name="psxx", bufs=4, space="PSUM") as p_psxx, \
                 tc.tile_pool(name="psa", bufs=4, space="PSUM") as p_psa:
                # all squares hoisted: DVE never blocks them behind the
                # per-quarter mxx split chains
                xsqs = {}
                for q4 in range(N // NQH):
                    qsl = slice(q4 * NQH, (q4 + 1) * NQH)
                    for c in range(CH):
                        xsq = p_a1.tile([CP, NQH], f16, tag="xsq", bufs=6)
                        nc.vector.tensor_mul(xsq[:], xhs[:, c, qsl],
                                             xhs[:, c, qsl])
                        xsqs[q4, c] = xsq
                for q4 in range(N // NQH):
                    qsl = slice(q4 * NQH, (q4 + 1) * NQH)
                    nts = [q4 * 2, q4 * 2 + 1]
                    psx = [p_psxx.tile([1, NT], f32, tag="psxx",
                                       name=f"psxx{q4}_{_j}")
                           for _j in range(2)]
                    pav = [p_psa.tile([128, NT], f32, tag="pa",
                                      name=f"pa{q4}_{_j}")
                           for _j in range(2)]
                    for c in range(CH):
                        xsq = xsqs[q4, c]
                        for j, nt in enumerate(nts):
                            mw = 128 if nt < NQ // NT else 64
                            nc.tensor.matmul(
                                psx[j][:], ones_col[:, :],
                                xsq[:, j * NT:(j + 1) * NT],
                                start=(c == 0), stop=(c == CH - 1))
                            nc.tensor.matmul(
                                pav[j][0:mw, :], w1s[:, c, 0:mw],
                                xhs[:, c, q4 * NQH + j * NT:
                                    q4 * NQH + (j + 1) * NT],
                                start=(c == 0), stop=(c == CH - 1))
                    mxxq = p_a1.tile([1, NQH], f32, tag="mxxq")
                    msp = p_a1.tile([1, NQH], f16, tag="msp")
                    for j, nt in enumerate(nts):
                        nc.scalar.mul(mxxq[:, j * NT:(j + 1) * NT],
                                      psx[j][:], -0.5)
                        nc.scalar.copy(A[:, nt * NT:(nt + 1) * NT],
                                       pav[j][0:64, :])
                        if nt < NQ // NT:
                            nc.scalar.copy(Bv[:, nt * NT:(nt + 1) * NT],
                                           pav[j][64:128, :])
                    # 2-way fp16 split of -xx/2 (split error ~1e-4) into
                    # chunk-7 pad partitions 104/105 (via DMA: engine
                    # start-partition must be 0/32/64/96)
                    for j in range(2):
                        nc.scalar.copy(msp[:], mxxq[:])
                        nc.sync.dma_start(
                            xhs[104 + j:105 + j, CH - 1, qsl], msp[:])
                        if j < 1:
                            nc.vector.tensor_sub(mxxq[:], mxxq[:], msp[:])

        # ---- phases B (knn) + D (convs) interleaved per segment ----
        with tc.tile_pool(name="pss", bufs=6, space="PSUM") as p_pss, \
             tc.tile_pool(name="psd", bufs=2, space="PSUM") as p_psd, \
             tc.tile_pool(name="ms", bufs=1) as p_s, \
             tc.tile_pool(name="m8", bufs=2) as p_m8, \
             tc.tile_pool(name="gq", bufs=2) as p_gq, \
             tc.tile_pool(name="seg", bufs=1) as p_seg, \
             tc.tile_pool(name="tmp", bufs=2) as p_tmp, \
             tc.tile_pool(name="osb", bufs=4) as p_osb:
            idxw = p_s.tile([64, 3 * NQ // 16], i16, tag="idxw")

            def knn_qt(qt):
                qs = slice(qt * QT, (qt + 1) * QT)
                # per key-tile top-8 straight from PSUM (overlaps the
                # remaining matmuls), then a 64-candidate merge: the
                # serial tail is ~1us instead of a 4.4us full-row scan
                m64 = p_m8.tile([QT, 64], f32, tag="m64")
                i64 = p_m8.tile([QT, 64], u16, tag="i64")
                for nt in range(NNT):
                    ns = slice(nt * NT, (nt + 1) * NT)
                    ps = p_pss.tile([QT, NT], f32, tag="pss")
                    for c in range(CH):
                        stat = xq7[:, qs] if c == CH - 1 else xhs[:, c, qs]
                        nc.tensor.matmul(ps[:], stat, xhs[:, c, ns],
                                         start=(c == 0), stop=(c == CH - 1))
                    c8 = slice(nt * 8, (nt + 1) * 8)
                    nc.vector.max(out=m64[:, c8], in_=ps[:])
                    nc.vector.max_index(out=i64[:, c8], in_max=m64[:, c8],
                                        in_values=ps[:])
                # global candidate indices as f32 values
                g64 = p_m8.tile([QT, 64], f32, tag="g64")
                nc.vector.tensor_copy(g64[:], i64[:])
                nc.vector.tensor_add(g64[:], g64[:], offs64[:])
                m8 = p_m8.tile([QT, 8], f32, tag="m8")
                nc.vector.max(out=m8[:], in_=m64[:])
                # per-partition index extraction: for each top-k value,
                # idx = sum((m64 == v_k) * g64) over the 64 candidates
                idx3 = p_m8.tile([QT, 3], f32, tag="idx3")
                eqm = p_m8.tile([QT, 64], f32, tag="eqm")
                for kk in range(3):
                    nc.vector.scalar_tensor_tensor(
                        out=eqm[:], in0=m64[:], scalar=m8[:, kk:kk + 1],
                        in1=g64[:], op0=mybir.AluOpType.is_equal,
                        op1=mybir.AluOpType.mult,
                        accum_out=idx3[:, kk:kk + 1])
                nc.vector.tensor_copy(stage3[:, qt, :], idx3[:])
                # wrap this qt's indices into ap_gather layout:
                # idxw[16*g2+r, qt*24 + g*3 + kk] = stage3[g*16+r, qt, kk].
                # For the last qt (critical tail) use one wave of 32
                # independent DMAs; otherwise 8 + 3 dependent replications
                # (less queue traffic, latency hidden by the next tile).
                if qt == NQT - 1:
                    for g2 in range(4):
                        for g in range(8):
                            nc.sync.dma_start(
                                idxw[16 * g2:16 * (g2 + 1),
                                     qt * 24 + 3 * g:qt * 24 + 3 * g + 3],
                                stage3[16 * g:16 * (g + 1), qt, :].bitcast(i16))
                else:
                    for g in range(8):
                        nc.sync.dma_start(
                            idxw[0:16, qt * 24 + 3 * g:qt * 24 + 3 * g + 3],
                            stage3[16 * g:16 * (g + 1), qt, :].bitcast(i16))
                    for g2 in range(1, 4):
                        nc.sync.dma_start(
                            idxw[16 * g2:16 * (g2 + 1), qt * 24:(qt + 1) * 24],
                            idxw[0:16, qt * 24:(qt + 1) * 24])
                # gather this qt's neighbor features (overlaps the
                # remaining distance matmuls on the Q7s);
                # gather position 16*(g*3+kk)+r = query g*16+r
                gq = p_gq.tile([64, 3 * QT], f32, tag="gq")
                nc.gpsimd.ap_gather(
                    out_ap=gq[:], in_ap=A[:],
                    idxs_ap=idxw[:, qt * 24:(qt + 1) * 24],
                    channels=64, num_elems=N, d=1, num_idxs=3 * QT)
                # unpermute into h1pre (fp16, kk-major q-order)
                gqv = gq.rearrange("p (g kk r) -> p g kk r",
                                   g=8, kk=3, r=16)
                hv = h1pre.rearrange("p (kk q) -> p kk q", kk=3)
                dst = hv[:, :, qt * QT:(qt + 1) * QT] \
                    .rearrange("p kk (g r) -> p g kk r", g=8)
                nc.scalar.copy(dst, gqv[:])

            def h1_seg(q0, w):
                # h1 = relu(bn1(h1pre + Bv)) in place for this segment
                qs = slice(q0, q0 + w)
                bvb = Bv[:, qs].unsqueeze(1).to_broadcast([64, 3, w])
                nc.vector.tensor_add(h1[:, :, qs], h1[:, :, qs], bvb)
                nc.scalar.activation(h1[:, :, qs], h1[:, :, qs], Relu,
                                     bias=sb1s[:, 1:2], scale=sb1s[:, 0:1])

            def conv_seg(q0, w):
                qs = slice(q0, q0 + w)
                h2 = p_seg.tile([128, 3, w], f16, tag="h2")
                h3 = p_seg.tile([128, 2, 3, w], f16, tag="h3")
                h4 = p_seg.tile([128, 4, 3, w], f16, tag="h4")
                cat = p_seg.tile([128, 8, w], f16, tag="cat")
                nc.vector.memset(cat[64:128, 0, :], 0.0)

                # conv2 (K=64 -> 128)
                for kk in range(3):
                    ps2 = p_psd.tile([128, w], f32, tag="psd")
                    nc.tensor.matmul(ps2[:], w2s[:], h1[:, kk, qs],
                                     start=True, stop=True)
                    nc.scalar.activation(h2[:, kk, :], ps2[:], Relu,
                                         bias=sb2s[:, 1:2], scale=sb2s[:, 0:1])
                # x1 -> cat chunk 0 (64 rows)
                t1 = p_tmp.tile([64, w], f16, tag="t64")
                nc.vector.tensor_max(t1[:], h1[:, 1, qs], h1[:, 2, qs])
                nc.vector.tensor_max(cat[0:64, 0, :], t1[:], h1[:, 0, qs])
                # x2 -> cat chunk 1
                t2 = p_tmp.tile([128, w], f16, tag="t128")
                nc.vector.tensor_max(t2[:], h2[:, 1, :], h2[:, 2, :])
                nc.vector.tensor_max(cat[:, 1, :], t2[:], h2[:, 0, :])

                # conv3 (K=128 -> 256 in 2 chunks)
                for m in range(2):
                    for kk in range(3):
                        ps3 = p_psd.tile([128, w], f32, tag="psd")
                        nc.tensor.matmul(ps3[:], w3s[:, m * 128:(m + 1) * 128],
                                         h2[:, kk, :], start=True, stop=True)
                        nc.scalar.activation(h3[:, m, kk, :], ps3[:], Relu,
                                             bias=sb3s[:, 2 + m:3 + m],
                                             scale=sb3s[:, m:m + 1])
                # x3 -> cat chunks 2,3
                for m in range(2):
                    t3 = p_tmp.tile([128, w], f16, tag="t128")
                    nc.vector.tensor_max(t3[:], h3[:, m, 1, :], h3[:, m, 2, :])
                    nc.vector.tensor_max(cat[:, 2 + m, :], t3[:], h3[:, m, 0, :])

                # conv4 (K=256 in 2 chunks -> 512 in 4 chunks)
                for m in range(4):
                    for kk in range(3):
                        ps4 = p_psd.tile([128, w], f32, tag="psd")
                        for c in range(2):
                            nc.tensor.matmul(
                                ps4[:], w4s[:, c, m * 128:(m + 1) * 128],
                                h3[:, c, kk, :], start=(c == 0), stop=(c == 1))
                        nc.scalar.activation(h4[:, m, kk, :], ps4[:], Relu,
                                             bias=sb4s[:, 4 + m:5 + m],
                                             scale=sb4s[:, m:m + 1])
                # x4 -> cat chunks 4..7
                for m in range(4):
                    t4 = p_tmp.tile([128, w], f16, tag="t128")
                    nc.vector.tensor_max(t4[:], h4[:, m, 1, :], h4[:, m, 2, :])
                    nc.vector.tensor_max(cat[:, 4 + m, :], t4[:], h4[:, m, 0, :])

                # conv5 (K=960 padded to 8*128 -> 1024 in 8 chunks)
                for m in range(8):
                    ps5 = p_psd.tile([128, w], f32, tag="psd")
                    for c in range(8):
                        nc.tensor.matmul(
                            ps5[:], w5s[:, c, m * 128:(m + 1) * 128],
                            cat[:, c, :], start=(c == 0), stop=(c == 7))
                    o5 = p_osb.tile([128, w], f32, tag="o5")
                    nc.scalar.activation(o5[:], ps5[:], Relu,
                                         bias=sb5s[:, 8 + m:9 + m],
                                         scale=sb5s[:, m:m + 1])
                    nc.sync.dma_start(outr[:, m, qs], o5[:])

            # emission schedule: each qt's h1 BN lags one full tile
            # window behind its gather chain (so the busy DVE queue
            # never waits on it); conv strips lag one qt behind their
            # last query tile; the last segment runs as two 256-col
            # strips to shrink the serial tail after the final knn tile
            conv_at = {4: (0, SEG), 8: (SEG, SEG), 12: (2 * SEG, SEG),
                       14: (3 * SEG, SEG // 2)}
            for qt in range(NQT):
                with nc.named_scope("knn"):
                    knn_qt(qt)
                if qt >= 1:
                    with nc.named_scope("gather"):
                        h1_seg((qt - 1) * QT, QT)
                if qt in conv_at:
                    with nc.named_scope("convs"):
                        conv_seg(*conv_at[qt])
            with nc.named_scope("gather"):
                h1_seg((NQT - 1) * QT, QT)
            with nc.named_scope("convs"):
                conv_seg(3 * SEG + SEG // 2, SEG // 2)


def prep_inputs(inputs):
    """Host-side sharding + layout/precision prep. Returns per-core in_maps."""
    x = np.ascontiguousarray(inputs["x"], dtype=np.float32)  # [B, C, N]
    shared = {}
    w1 = inputs["w1"].astype(np.float32)
    w1p = np.zeros((CPAD, 128), dtype=np.float16)
    w1p[:C_IN, 0:64] = w1[:, :C_IN].T.astype(np.float16)
    w1p[:C_IN, 64:128] = w1[:, C_IN:].T.astype(np.float16)
    shared["w1t"] = w1p
    shared["w2t"] = np.ascontiguousarray(inputs["w2"].T.astype(np.float16))
    shared["w3t"] = np.ascontiguousarray(inputs["w3"].T.astype(np.float16))
    shared["w4t"] = np.ascontiguousarray(inputs["w4"].T.astype(np.float16))
    w5t = inputs["w5"].astype(np.float32).T  # [960, 1024]
    w5p = np.zeros((128, 8, 1024), dtype=np.float16)
    w5p[0:64, 0, :] = w5t[0:64]          # x1 block
    w5p[:, 1, :] = w5t[64:192]           # x2
    w5p[:, 2, :] = w5t[192:320]          # x3 lo
    w5p[:, 3, :] = w5t[320:448]          # x3 hi
    for m in range(4):                   # x4
        w5p[:, 4 + m, :] = w5t[448 + 128 * m:448 + 128 * (m + 1)]
    shared["w5p"] = w5p

    def scale_bias(i):
        g = inputs[f"g{i}"].astype(np.float32)
        b = inputs[f"b{i}"].astype(np.float32)
        m = inputs[f"m{i}"].astype(np.float32)
        v = inputs[f"v{i}"].astype(np.float32)
        s = g / np.sqrt(v + EPS)
        return s.astype(np.float32), (b - m * s).astype(np.float32)

    s1, b1 = scale_bias(1)
    shared["sb1"] = np.ascontiguousarray(np.stack([s1, b1], axis=1))
    s2, b2 = scale_bias(2)
    shared["sb2"] = np.ascontiguousarray(np.stack([s2, b2], axis=1))
    s3, b3 = scale_bias(3)
    shared["sb3"] = np.ascontiguousarray(
        np.stack([s3[:128], s3[128:], b3[:128], b3[128:]], axis=1))
    s4, b4 = scale_bias(4)
    shared["sb4"] = np.ascontiguousarray(np.stack(
        [s4[128 * m:128 * (m + 1)] for m in range(4)]
        + [b4[128 * m:128 * (m + 1)] for m in range(4)], axis=1))
    s5, b5 = scale_bias(5)
    shared["sb5"] = np.ascontiguousarray(np.stack(
        [s5[128 * m:128 * (m + 1)] for m in range(8)]
        + [b5[128 * m:128 * (m + 1)] for m in range(8)], axis=1))

    in_maps = []
    for core in range(8):
        b, half = core // 2, core % 2
        q0 = half * NQ
        other0 = NQ - q0  # 2048 if half==0 else 0
        xbp = np.concatenate([x[b][:, q0:q0 + NQ], x[b][:, other0:other0 + NQ]],
                             axis=1)  # [1000, 4096], own queries first
        xpad = np.zeros((CPAD, N), dtype=np.float16)
        xpad[:C_IN] = xbp.astype(np.float16)
        m = dict(shared)
        m["xh"] = xpad
        in_maps.append(m)
    return in_maps


def kernel(**inputs):
    from concourse.bass_utils import run_bass_kernel_spmd

    if "nc" not in _CACHE:
        _CACHE["nc"] = build_nc()
    nc = _CACHE["nc"]
    in_maps = prep_inputs(inputs)
    res = run_bass_kernel_spmd(nc, in_maps, core_ids=list(range(8)))
    out = np.empty((B, 1024, N), dtype=np.float32)
    for core in range(8):
        b, half = core // 2, core % 2
        q0 = half * NQ
        out[b, :, q0:q0 + NQ] = res.results[core]["out"]
    return out

